# revision 25
# baseline (speedup 1.0000x reference)
"""Bahdanau additive attention between two sequences on 8 Trainium2 NeuronCores.

Reference computation (per batch b):
    s0 = q0 @ W1[:D]          # [L, O]
    s1 = q1 @ W1[D:]          # [L, O]
    h[i,j,:]   = tanh(s1[i] + s0[j] + b1)          # [L, L, O]
    attn[i,j]  = h[i,j,:] . W2 (+ b2, drops out of softmax)
    masked     = attn + -1e8 * mask0[i]*mask1[j]
    a_m1 = softmax(masked over j);  a_m2 = softmax(masked over i)
    out0[i] = sum_j a_m1[i,j] q1[j];  out1[j] = sum_i a_m2[i,j] q0[i]

Sharding: data-parallel over batch B=16 -> 2 batches per core; params replicated.

Device strategy per (batch, oc) chunk of 128 output-features:
  PE:  s0T/s1T = W1-chunk.T @ q0T/q1T (fp32), then scores += W2oc.T @ tanh(...)
       as a [128,1]x[128,512] matvec in float32r (full rate at N=512),
       accumulated over the 8 oc chunks in PSUM.
  DVE/GPSIMD: outer-sum tmp[o',(i,j)] = s0T[o',j] + s1T[o',i] via
       step-0 broadcast APs (the dominant vector cost, split across engines).
  ACT: h = tanh(tmp + b1[o'] per-partition bias) - the dominant cost
       (~8.4M transcendentals per core).
Epilogue per batch: exp on ACT, mask multiply (host-precomputed 1-m0*m1),
row/col sums via DVE reduce + PE transpose, normalization folded into the
final bmms as per-partition scales.
"""

import numpy as np

B, L, D, O = 16, 64, 512, 1024
N_CORES = 8
BPC = B // N_CORES  # batches per core
OC = O // 128  # 8 o-chunks
DC = D // 128  # 4 d-chunks
NT = (L * L) // 512  # 8 ij-tiles of 512 per batch

PE_ADD_OC = (0,)  # o-chunks whose outer-add runs on PE (PE idle at batch start)

_CACHE = {}


def _build_nc(debug=False):
    import concourse.mybir as mybir
    import concourse.tile as tile
    from concourse import bacc
    from concourse.masks import make_identity

    f32 = mybir.dt.float32
    f32r = mybir.dt.float32r
    bf16 = mybir.dt.bfloat16
    AF = mybir.ActivationFunctionType

    nc = bacc.Bacc("TRN2", target_bir_lowering=False)

    q0n = nc.dram_tensor("q0n", [BPC, L, D], f32r, kind="ExternalInput")
    q1n = nc.dram_tensor("q1n", [BPC, L, D], f32r, kind="ExternalInput")
    q0t = nc.dram_tensor("q0t", [BPC, D, L], bf16, kind="ExternalInput")
    q1t = nc.dram_tensor("q1t", [BPC, D, L], bf16, kind="ExternalInput")
    wm = nc.dram_tensor("wm", [BPC, L, L], f32, kind="ExternalInput")
    w1 = nc.dram_tensor("w1", [2 * D, O], bf16, kind="ExternalInput")
    b1t = nc.dram_tensor("b1t", [128, OC], f32, kind="ExternalInput")
    w2t = nc.dram_tensor("w2t", [128, OC], bf16, kind="ExternalInput")
    out0 = nc.dram_tensor("out0", [BPC, L, D], f32, kind="ExternalOutput")
    out1 = nc.dram_tensor("out1", [BPC, L, D], f32, kind="ExternalOutput")
    if debug:
        dbg_attn = nc.dram_tensor("dbg_attn", [BPC, L, L], f32, kind="ExternalOutput")
        dbg_em = nc.dram_tensor("dbg_em", [BPC, L, L], f32, kind="ExternalOutput")
        dbg_st = nc.dram_tensor("dbg_st", [BPC, OC, 128, 128], f32, kind="ExternalOutput")
        dbg_h = nc.dram_tensor("dbg_h", [BPC, 128, L, L], f32, kind="ExternalOutput")
        dbg_w1 = nc.dram_tensor("dbg_w1", [128, 2 * DC, O], f32, kind="ExternalOutput")
        dbg_qt = nc.dram_tensor("dbg_qt", [128, BPC, DC, L], f32, kind="ExternalOutput")

    with tile.TileContext(nc) as tc:
        with (
            tc.tile_pool(name="consts", bufs=1) as consts,
            tc.tile_pool(name="qdata", bufs=BPC) as qdata,
            tc.tile_pool(name="st", bufs=3) as st_pool,
            tc.tile_pool(name="tmp", bufs=3) as tmp_pool,
            tc.tile_pool(name="hbuf", bufs=3) as h_pool,
            tc.tile_pool(name="ep", bufs=2) as ep_pool,
            tc.tile_pool(name="outb", bufs=2) as out_pool,
            tc.tile_pool(name="ps_scores", bufs=1, space="PSUM") as ps_scores,
            tc.tile_pool(name="ps_st", bufs=1, space="PSUM") as ps_st,
            tc.tile_pool(name="ps_ep", bufs=1, space="PSUM") as ps_ep,
            tc.tile_pool(name="ps_add", bufs=2, space="PSUM") as ps_add,
            tc.tile_pool(name="ps_tr", bufs=1, space="PSUM") as ps_tr,
        ):
            # ---- constants / weights ----
            # W1 loaded per o-chunk so chunk 0 can start after ~256KB, not 2MB
            w1s = consts.tile([128, OC, 2 * DC, 128], bf16)
            for occ in range(OC):
                nc.sync.dma_start(
                    out=w1s[:, occ, :, :],
                    in_=w1[:, occ * 128 : (occ + 1) * 128].rearrange(
                        "(c p) o -> p c o", p=128
                    ),
                )
            b1s = consts.tile([128, OC], f32)
            nc.sync.dma_start(out=b1s[:], in_=b1t[:])
            w2s = consts.tile([128, OC], bf16)
            nc.sync.dma_start(out=w2s[:], in_=w2t[:])
            ident = consts.tile([128, 128], f32)
            make_identity(nc, ident[:])
            ident_r = consts.tile([128, 128], f32r)
            nc.vector.tensor_copy(ident_r[:], ident[:])

            # ---- per-batch inputs (one DMA per tensor, all batches) ----
            q0ts_all = qdata.tile([128, BPC, DC, L], bf16, tag="q0ts", bufs=1)
            q1ts_all = qdata.tile([128, BPC, DC, L], bf16, tag="q1ts", bufs=1)
            for b in range(BPC):
                nc.sync.dma_start(
                    out=q0ts_all[:, b, :, :],
                    in_=q0t[b].rearrange("(c p) j -> p c j", p=128),
                )
                nc.sync.dma_start(
                    out=q1ts_all[:, b, :, :],
                    in_=q1t[b].rearrange("(c p) j -> p c j", p=128),
                )
            # epilogue-only tensors: load on the gpsimd (SWDGE) queue so they
            # don't compete with W1/q-transpose on the sync queue at startup
            q0ns_all = qdata.tile([L, BPC, D], f32r, tag="q0ns", bufs=1)
            nc.gpsimd.dma_start(out=q0ns_all[:], in_=q0n.rearrange("b l d -> l b d"))
            q1ns_all = qdata.tile([L, BPC, D], f32r, tag="q1ns", bufs=1)
            nc.gpsimd.dma_start(out=q1ns_all[:], in_=q1n.rearrange("b l d -> l b d"))
            wms_all = qdata.tile([L, BPC, L], f32, tag="wms", bufs=1)
            nc.gpsimd.dma_start(out=wms_all[:], in_=wm.rearrange("b l j -> l b j"))
            if debug:
                w1f_d = qdata.tile([128, OC, 2 * DC, 128], f32, tag="w1f_d", bufs=1)
                nc.vector.tensor_copy(w1f_d[:], w1s[:])
                nc.sync.dma_start(out=dbg_w1[:], in_=w1f_d[:])
                qt_d = qdata.tile([128, BPC, DC, L], f32, tag="qt_d", bufs=1)
                nc.vector.tensor_copy(qt_d[:], q0ts_all[:])
                nc.sync.dma_start(out=dbg_qt[:], in_=qt_d[:])
            q0ts = [q0ts_all[:, b, :, :] for b in range(BPC)]
            q1ts = [q1ts_all[:, b, :, :] for b in range(BPC)]
            q0ns = [q0ns_all[:, b, :] for b in range(BPC)]
            q1ns = [q1ns_all[:, b, :] for b in range(BPC)]
            wms = [wms_all[:, b, :] for b in range(BPC)]

            # scores accumulators: 8 matvec chains per batch. The bf16 matmul
            # path supports PSUM column groups {0, 32, 64} (4-byte matmuls are
            # partition-0 only, and base 96 is rejected), so pack 3 chains per
            # bank -> 3 banks per batch.
            scores = [
                [
                    ps_scores.tile(
                        [65, 512], f32, tag=f"scores_g{g}", name=f"scores{b}_g{g}"
                    )
                    for g in range(3)
                ]
                for b in range(BPC)
            ]

            def score_chain(b, t):
                return scores[b][t // 3][32 * (t % 3) : 32 * (t % 3) + 1, :]

            # ---- main pipeline ----
            pending = None
            for b in range(BPC):
                for oc in range(OC):
                    osl = slice(oc * 128, (oc + 1) * 128)
                    # s0T / s1T for this o-chunk: PSUM [128, 128]
                    st_ps = ps_st.tile([128, 128], f32, tag="st_ps")
                    for dc in range(DC):
                        nc.tensor.matmul(
                            out=st_ps[:, 0:64],
                            lhsT=w1s[:, oc, dc, :],
                            rhs=q0ts[b][:, dc, :],
                            start=(dc == 0),
                            stop=(dc == DC - 1),
                        )
                    for dc in range(DC):
                        nc.tensor.matmul(
                            out=st_ps[:, 64:128],
                            lhsT=w1s[:, oc, DC + dc, :],
                            rhs=q1ts[b][:, dc, :],
                            start=(dc == 0),
                            stop=(dc == DC - 1),
                        )
                    st_sb = st_pool.tile([128, 128], f32r, tag="st_sb")
                    if oc % 2 == 0:
                        nc.scalar.copy(st_sb[:], st_ps[:])
                    else:
                        nc.vector.tensor_copy(st_sb[:], st_ps[:])
                    if debug:
                        nc.sync.dma_start(out=dbg_st[b, oc], in_=st_sb[:].bitcast(f32))

                    # outer sum tmp[o', (i, j)] = s0T[o', j] + s1T[o', i].
                    # GPSIMD shares SBUF ports with DVE (no net win), so split
                    # between DVE (tensor_add) and PE (identity-matmul into
                    # PSUM, f32r at full rate; PE has its own SBUF ports).
                    in_j = st_sb[:, 0:64].unsqueeze(1).broadcast_to((128, L, L))
                    in_i = st_sb[:, 64:128].unsqueeze(2).broadcast_to((128, L, L))
                    h3 = h_pool.tile([128, L * L], bf16, tag="h3")
                    if oc in PE_ADD_OC:
                        for n in range(NT):
                            nsl = slice(n * 8, (n + 1) * 8)
                            ptmp = ps_add.tile([128, 512], f32, tag="ptmp")
                            nc.tensor.matmul(
                                out=ptmp[:],
                                lhsT=ident_r[:],
                                rhs=in_j[:, nsl, :],
                                start=True,
                                stop=False,
                            )
                            nc.tensor.matmul(
                                out=ptmp[:],
                                lhsT=ident_r[:],
                                rhs=in_i[:, nsl, :],
                                start=False,
                                stop=True,
                            )
                            nc.scalar.activation(
                                h3[:, n * 512 : (n + 1) * 512],
                                ptmp[:],
                                AF.Tanh,
                                bias=b1s[:, oc : oc + 1],
                            )
                    else:
                        tmp3 = tmp_pool.tile([128, L * L], f32, tag="tmp3")
                        tmp3_3d = tmp3[:].rearrange("p (i j) -> p i j", i=L)
                        nc.vector.tensor_add(tmp3_3d, in_j, in_i)
                        nc.scalar.activation(
                            h3[:], tmp3[:], AF.Tanh, bias=b1s[:, oc : oc + 1]
                        )

                    if debug and oc == 0:
                        h32 = st_pool.tile([128, L * L], f32, tag="h32", name=f"h32_{b}")
                        nc.vector.tensor_copy(h32[:], h3[:])
                        nc.sync.dma_start(out=dbg_h[b], in_=h32[:].rearrange("p (i j) -> p i j", i=L))
                    # matvec for the PREVIOUS chunk (software pipelining: PE
                    # runs chunk c's s-matmuls while ACT finishes tanh(c))
                    if pending is not None:
                        pb, poc, ph = pending
                        for t in range(NT):
                            nc.tensor.matmul(
                                out=score_chain(pb, t),
                                lhsT=w2s[:, poc : poc + 1],
                                rhs=ph[:, t * 512 : (t + 1) * 512],
                                start=(poc == 0),
                                stop=(poc == OC - 1),
                            )
                    pending = (b, oc, h3)

                # flush the last chunk's matvec before batch b's epilogue
                pb, poc, ph = pending
                for t in range(NT):
                    nc.tensor.matmul(
                        out=score_chain(pb, t),
                        lhsT=w2s[:, poc : poc + 1],
                        rhs=ph[:, t * 512 : (t + 1) * 512],
                        start=(poc == 0),
                        stop=(poc == OC - 1),
                    )
                pending = None

                # ---- epilogue for batch b ----
                # PSUM can't be DMA'd: bounce score banks through SBUF (cost is
                # free-dim driven, so copying the full partition range is cheap)
                stg = []
                for g in range(3):
                    sg = ep_pool.tile([65, 512], f32, tag="stg", name=f"stg{b}_{g}")
                    nc.scalar.copy(sg[:], scores[b][g][:])
                    stg.append(sg)
                attn = ep_pool.tile([L, L], f32, tag="attn")
                for t in range(NT):
                    src = stg[t // 3][32 * (t % 3) : 32 * (t % 3) + 1, :]
                    nc.sync.dma_start(
                        out=attn[t * 8 : (t + 1) * 8, :],
                        in_=src,
                    )
                if debug:
                    nc.sync.dma_start(out=dbg_attn[b], in_=attn[:])
                ex = ep_pool.tile([L, L], f32, tag="ex")
                nc.scalar.activation(ex[:], attn[:], AF.Exp)
                em = ep_pool.tile([L, L], f32, tag="em")
                nc.vector.tensor_mul(em[:], ex[:], wms[b])

                if debug:
                    nc.sync.dma_start(out=dbg_em[b], in_=em[:])
                rs = ep_pool.tile([L, 1], f32, tag="rs")
                nc.vector.reduce_sum(rs[:], em[:], axis=mybir.AxisListType.X)
                rrecip = ep_pool.tile([L, 1], f32, tag="rrecip")
                nc.vector.reciprocal(rrecip[:], rs[:])

                emt_ps = ps_tr.tile([L, L], f32, tag="emt_ps")
                nc.tensor.transpose(emt_ps[:], em[:], ident[0:L, 0:L])
                emt = ep_pool.tile([L, L], f32r, tag="emt")
                nc.scalar.copy(emt[:], emt_ps[:])
                em_r = ep_pool.tile([L, L], f32r, tag="em_r")
                nc.vector.tensor_copy(em_r[:], em[:])

                cs = ep_pool.tile([L, 1], f32, tag="cs")
                nc.vector.reduce_sum(cs[:], emt[:], axis=mybir.AxisListType.X)
                crecip = ep_pool.tile([L, 1], f32, tag="crecip")
                nc.vector.reciprocal(crecip[:], cs[:])

                # out0[i, d] = rrecip[i] * sum_j em[i, j] q1[j, d]
                o0_ps = ps_ep.tile([L, D], f32, tag="o_ps")
                nc.tensor.matmul(
                    out=o0_ps[:],
                    lhsT=emt[:],
                    rhs=q1ns[b],
                    start=True,
                    stop=True,
                )
                o0_sb = out_pool.tile([L, D], f32, tag="o0_sb")
                nc.scalar.activation(o0_sb[:], o0_ps[:], AF.Copy, scale=rrecip[:])
                nc.sync.dma_start(out=out0[b], in_=o0_sb[:])

                # out1[j, d] = crecip[j] * sum_i em[i, j] q0[i, d]
                o1_ps = ps_ep.tile([L, D], f32, tag="o_ps")
                nc.tensor.matmul(
                    out=o1_ps[:],
                    lhsT=em_r[:],
                    rhs=q0ns[b],
                    start=True,
                    stop=True,
                )
                o1_sb = out_pool.tile([L, D], f32, tag="o1_sb")
                nc.scalar.activation(o1_sb[:], o1_ps[:], AF.Copy, scale=crecip[:])
                nc.sync.dma_start(out=out1[b], in_=o1_sb[:])

    nc.finalize()
    return nc


def _get_nc():
    if "nc" not in _CACHE:
        _CACHE["nc"] = _build_nc()
    return _CACHE["nc"]


def kernel(q0, q1, mask0, mask1, W1, b1, W2, b2, **_unused):
    from concourse.bass_utils import run_bass_kernel_spmd

    q0 = np.asarray(q0, dtype=np.float32)
    q1 = np.asarray(q1, dtype=np.float32)
    W1 = np.ascontiguousarray(np.asarray(W1, dtype=np.float32))
    b1 = np.asarray(b1, dtype=np.float32)
    W2 = np.asarray(W2, dtype=np.float32)
    m0f = np.asarray(mask0).astype(np.float32)
    m1f = np.asarray(mask1).astype(np.float32)

    # host-side prep (tiny): transposed q views, mask outer product, param tiling
    wm_full = (1.0 - m0f[:, :, None] * m1f[:, None, :]).astype(np.float32)
    b1t = np.ascontiguousarray(b1.reshape(OC, 128).T)
    import ml_dtypes

    w2t = np.ascontiguousarray(W2[:, 0].reshape(OC, 128).T).astype(ml_dtypes.bfloat16)
    W1bf = W1.astype(ml_dtypes.bfloat16)

    in_maps = []
    for c in range(N_CORES):
        sl = slice(BPC * c, BPC * (c + 1))
        q0c = np.ascontiguousarray(q0[sl])
        q1c = np.ascontiguousarray(q1[sl])
        in_maps.append(
            {
                "q0n": q0c,
                "q1n": q1c,
                "q0t": np.ascontiguousarray(q0c.transpose(0, 2, 1)).astype(
                    ml_dtypes.bfloat16
                ),
                "q1t": np.ascontiguousarray(q1c.transpose(0, 2, 1)).astype(
                    ml_dtypes.bfloat16
                ),
                "wm": np.ascontiguousarray(wm_full[sl]),
                "w1": W1bf,
                "b1t": b1t,
                "w2t": w2t,
            }
        )

    nc = _get_nc()
    res = run_bass_kernel_spmd(nc, in_maps, core_ids=list(range(N_CORES)))
    out0 = np.concatenate([res.results[c]["out0"] for c in range(N_CORES)], axis=0)
    out1 = np.concatenate([res.results[c]["out1"] for c in range(N_CORES)], axis=0)
    return out0, out1


# revision 26
# speedup vs baseline: 1.0029x; 1.0029x over previous
"""Bahdanau additive attention between two sequences on 8 Trainium2 NeuronCores.

Reference computation (per batch b):
    s0 = q0 @ W1[:D]          # [L, O]
    s1 = q1 @ W1[D:]          # [L, O]
    h[i,j,:]   = tanh(s1[i] + s0[j] + b1)          # [L, L, O]
    attn[i,j]  = h[i,j,:] . W2 (+ b2, drops out of softmax)
    masked     = attn + -1e8 * mask0[i]*mask1[j]
    a_m1 = softmax(masked over j);  a_m2 = softmax(masked over i)
    out0[i] = sum_j a_m1[i,j] q1[j];  out1[j] = sum_i a_m2[i,j] q0[i]

Sharding: data-parallel over batch B=16 -> 2 batches per core; params replicated.

Device strategy per (batch, oc) chunk of 128 output-features:
  PE:  s0T/s1T = W1-chunk.T @ q0T/q1T (fp32), then scores += W2oc.T @ tanh(...)
       as a [128,1]x[128,512] matvec in float32r (full rate at N=512),
       accumulated over the 8 oc chunks in PSUM.
  DVE/GPSIMD: outer-sum tmp[o',(i,j)] = s0T[o',j] + s1T[o',i] via
       step-0 broadcast APs (the dominant vector cost, split across engines).
  ACT: h = tanh(tmp + b1[o'] per-partition bias) - the dominant cost
       (~8.4M transcendentals per core).
Epilogue per batch: exp on ACT, mask multiply (host-precomputed 1-m0*m1),
row/col sums via DVE reduce + PE transpose, normalization folded into the
final bmms as per-partition scales.
"""

import numpy as np

B, L, D, O = 16, 64, 512, 1024
N_CORES = 8
BPC = B // N_CORES  # batches per core
OC = O // 128  # 8 o-chunks
DC = D // 128  # 4 d-chunks
NT = (L * L) // 512  # 8 ij-tiles of 512 per batch

PE_ADD_OC = ()  # outer-adds all on DVE

_CACHE = {}


def _build_nc(debug=False):
    import concourse.mybir as mybir
    import concourse.tile as tile
    from concourse import bacc
    from concourse.masks import make_identity

    f32 = mybir.dt.float32
    f32r = mybir.dt.float32r
    bf16 = mybir.dt.bfloat16
    AF = mybir.ActivationFunctionType

    nc = bacc.Bacc("TRN2", target_bir_lowering=False)

    q0n = nc.dram_tensor("q0n", [BPC, L, D], f32r, kind="ExternalInput")
    q1n = nc.dram_tensor("q1n", [BPC, L, D], f32r, kind="ExternalInput")
    q0t = nc.dram_tensor("q0t", [BPC, D, L], bf16, kind="ExternalInput")
    q1t = nc.dram_tensor("q1t", [BPC, D, L], bf16, kind="ExternalInput")
    wm = nc.dram_tensor("wm", [BPC, L, L], f32, kind="ExternalInput")
    w1 = nc.dram_tensor("w1", [2 * D, O], bf16, kind="ExternalInput")
    b1t = nc.dram_tensor("b1t", [128, OC], f32, kind="ExternalInput")
    w2t = nc.dram_tensor("w2t", [128, OC], bf16, kind="ExternalInput")
    out0 = nc.dram_tensor("out0", [BPC, L, D], f32, kind="ExternalOutput")
    out1 = nc.dram_tensor("out1", [BPC, L, D], f32, kind="ExternalOutput")
    if debug:
        dbg_attn = nc.dram_tensor("dbg_attn", [BPC, L, L], f32, kind="ExternalOutput")
        dbg_em = nc.dram_tensor("dbg_em", [BPC, L, L], f32, kind="ExternalOutput")
        dbg_st = nc.dram_tensor("dbg_st", [BPC, OC, 128, 128], f32, kind="ExternalOutput")
        dbg_h = nc.dram_tensor("dbg_h", [BPC, 128, L, L], f32, kind="ExternalOutput")
        dbg_w1 = nc.dram_tensor("dbg_w1", [128, 2 * DC, O], f32, kind="ExternalOutput")
        dbg_qt = nc.dram_tensor("dbg_qt", [128, BPC, DC, L], f32, kind="ExternalOutput")

    with tile.TileContext(nc) as tc:
        with (
            tc.tile_pool(name="consts", bufs=1) as consts,
            tc.tile_pool(name="qdata", bufs=BPC) as qdata,
            tc.tile_pool(name="st", bufs=3) as st_pool,
            tc.tile_pool(name="tmp", bufs=3) as tmp_pool,
            tc.tile_pool(name="hbuf", bufs=3) as h_pool,
            tc.tile_pool(name="ep", bufs=2) as ep_pool,
            tc.tile_pool(name="outb", bufs=2) as out_pool,
            tc.tile_pool(name="ps_scores", bufs=1, space="PSUM") as ps_scores,
            tc.tile_pool(name="ps_st", bufs=1, space="PSUM") as ps_st,
            tc.tile_pool(name="ps_ep", bufs=1, space="PSUM") as ps_ep,
            tc.tile_pool(name="ps_add", bufs=2, space="PSUM") as ps_add,
            tc.tile_pool(name="ps_tr", bufs=1, space="PSUM") as ps_tr,
        ):
            # ---- constants / weights ----
            # W1 loaded per o-chunk so chunk 0 can start after ~256KB, not 2MB
            w1s = consts.tile([128, OC, 2 * DC, 128], bf16)
            for occ in range(OC):
                nc.sync.dma_start(
                    out=w1s[:, occ, :, :],
                    in_=w1[:, occ * 128 : (occ + 1) * 128].rearrange(
                        "(c p) o -> p c o", p=128
                    ),
                )
            b1s = consts.tile([128, OC], f32)
            nc.sync.dma_start(out=b1s[:], in_=b1t[:])
            w2s = consts.tile([128, OC], bf16)
            nc.sync.dma_start(out=w2s[:], in_=w2t[:])
            ident = consts.tile([128, 128], f32)
            make_identity(nc, ident[:])
            ident_r = consts.tile([128, 128], f32r)
            nc.vector.tensor_copy(ident_r[:], ident[:])

            # ---- per-batch inputs (one DMA per tensor, all batches) ----
            q0ts_all = qdata.tile([128, BPC, DC, L], bf16, tag="q0ts", bufs=1)
            q1ts_all = qdata.tile([128, BPC, DC, L], bf16, tag="q1ts", bufs=1)
            for b in range(BPC):
                nc.sync.dma_start(
                    out=q0ts_all[:, b, :, :],
                    in_=q0t[b].rearrange("(c p) j -> p c j", p=128),
                )
                nc.sync.dma_start(
                    out=q1ts_all[:, b, :, :],
                    in_=q1t[b].rearrange("(c p) j -> p c j", p=128),
                )
            # epilogue-only tensors: load on the gpsimd (SWDGE) queue so they
            # don't compete with W1/q-transpose on the sync queue at startup
            q0ns_all = qdata.tile([L, BPC, D], f32r, tag="q0ns", bufs=1)
            nc.gpsimd.dma_start(out=q0ns_all[:], in_=q0n.rearrange("b l d -> l b d"))
            q1ns_all = qdata.tile([L, BPC, D], f32r, tag="q1ns", bufs=1)
            nc.gpsimd.dma_start(out=q1ns_all[:], in_=q1n.rearrange("b l d -> l b d"))
            wms_all = qdata.tile([L, BPC, L], f32, tag="wms", bufs=1)
            nc.gpsimd.dma_start(out=wms_all[:], in_=wm.rearrange("b l j -> l b j"))
            if debug:
                w1f_d = qdata.tile([128, OC, 2 * DC, 128], f32, tag="w1f_d", bufs=1)
                nc.vector.tensor_copy(w1f_d[:], w1s[:])
                nc.sync.dma_start(out=dbg_w1[:], in_=w1f_d[:])
                qt_d = qdata.tile([128, BPC, DC, L], f32, tag="qt_d", bufs=1)
                nc.vector.tensor_copy(qt_d[:], q0ts_all[:])
                nc.sync.dma_start(out=dbg_qt[:], in_=qt_d[:])
            q0ts = [q0ts_all[:, b, :, :] for b in range(BPC)]
            q1ts = [q1ts_all[:, b, :, :] for b in range(BPC)]
            q0ns = [q0ns_all[:, b, :] for b in range(BPC)]
            q1ns = [q1ns_all[:, b, :] for b in range(BPC)]
            wms = [wms_all[:, b, :] for b in range(BPC)]

            # scores accumulators: 8 matvec chains per batch. The bf16 matmul
            # path supports PSUM column groups {0, 32, 64} (4-byte matmuls are
            # partition-0 only, and base 96 is rejected), so pack 3 chains per
            # bank -> 3 banks per batch.
            scores = [
                [
                    ps_scores.tile(
                        [65, 512], f32, tag=f"scores_g{g}", name=f"scores{b}_g{g}"
                    )
                    for g in range(3)
                ]
                for b in range(BPC)
            ]

            def score_chain(b, t):
                return scores[b][t // 3][32 * (t % 3) : 32 * (t % 3) + 1, :]

            # ---- main pipeline ----
            pending = None
            for b in range(BPC):
                for oc in range(OC):
                    osl = slice(oc * 128, (oc + 1) * 128)
                    # s0T / s1T for this o-chunk: PSUM [128, 128]
                    st_ps = ps_st.tile([128, 128], f32, tag="st_ps")
                    for dc in range(DC):
                        nc.tensor.matmul(
                            out=st_ps[:, 0:64],
                            lhsT=w1s[:, oc, dc, :],
                            rhs=q0ts[b][:, dc, :],
                            start=(dc == 0),
                            stop=(dc == DC - 1),
                        )
                    for dc in range(DC):
                        nc.tensor.matmul(
                            out=st_ps[:, 64:128],
                            lhsT=w1s[:, oc, DC + dc, :],
                            rhs=q1ts[b][:, dc, :],
                            start=(dc == 0),
                            stop=(dc == DC - 1),
                        )
                    st_sb = st_pool.tile([128, 128], f32r, tag="st_sb")
                    if oc % 2 == 0:
                        nc.scalar.copy(st_sb[:], st_ps[:])
                    else:
                        nc.vector.tensor_copy(st_sb[:], st_ps[:])
                    if debug:
                        nc.sync.dma_start(out=dbg_st[b, oc], in_=st_sb[:].bitcast(f32))

                    # outer sum tmp[o', (i, j)] = s0T[o', j] + s1T[o', i].
                    # GPSIMD shares SBUF ports with DVE (no net win), so split
                    # between DVE (tensor_add) and PE (identity-matmul into
                    # PSUM, f32r at full rate; PE has its own SBUF ports).
                    in_j = st_sb[:, 0:64].unsqueeze(1).broadcast_to((128, L, L))
                    in_i = st_sb[:, 64:128].unsqueeze(2).broadcast_to((128, L, L))
                    h3 = h_pool.tile([128, L * L], bf16, tag="h3")
                    if oc in PE_ADD_OC:
                        for n in range(NT):
                            nsl = slice(n * 8, (n + 1) * 8)
                            ptmp = ps_add.tile([128, 512], f32, tag="ptmp")
                            nc.tensor.matmul(
                                out=ptmp[:],
                                lhsT=ident_r[:],
                                rhs=in_j[:, nsl, :],
                                start=True,
                                stop=False,
                            )
                            nc.tensor.matmul(
                                out=ptmp[:],
                                lhsT=ident_r[:],
                                rhs=in_i[:, nsl, :],
                                start=False,
                                stop=True,
                            )
                            nc.scalar.activation(
                                h3[:, n * 512 : (n + 1) * 512],
                                ptmp[:],
                                AF.Tanh,
                                bias=b1s[:, oc : oc + 1],
                            )
                    else:
                        tmp3 = tmp_pool.tile([128, L * L], f32, tag="tmp3")
                        tmp3_3d = tmp3[:].rearrange("p (i j) -> p i j", i=L)
                        nc.vector.tensor_add(tmp3_3d, in_j, in_i)
                        nc.scalar.activation(
                            h3[:], tmp3[:], AF.Tanh, bias=b1s[:, oc : oc + 1]
                        )

                    if debug and oc == 0:
                        h32 = st_pool.tile([128, L * L], f32, tag="h32", name=f"h32_{b}")
                        nc.vector.tensor_copy(h32[:], h3[:])
                        nc.sync.dma_start(out=dbg_h[b], in_=h32[:].rearrange("p (i j) -> p i j", i=L))
                    # matvec for the PREVIOUS chunk (software pipelining: PE
                    # runs chunk c's s-matmuls while ACT finishes tanh(c))
                    if pending is not None:
                        pb, poc, ph = pending
                        for t in range(NT):
                            nc.tensor.matmul(
                                out=score_chain(pb, t),
                                lhsT=w2s[:, poc : poc + 1],
                                rhs=ph[:, t * 512 : (t + 1) * 512],
                                start=(poc == 0),
                                stop=(poc == OC - 1),
                            )
                    pending = (b, oc, h3)

                # flush the last chunk's matvec before batch b's epilogue
                pb, poc, ph = pending
                for t in range(NT):
                    nc.tensor.matmul(
                        out=score_chain(pb, t),
                        lhsT=w2s[:, poc : poc + 1],
                        rhs=ph[:, t * 512 : (t + 1) * 512],
                        start=(poc == 0),
                        stop=(poc == OC - 1),
                    )
                pending = None

                # ---- epilogue for batch b ----
                # PSUM can't be DMA'd: bounce score banks through SBUF (cost is
                # free-dim driven, so copying the full partition range is cheap)
                stg = []
                for g in range(3):
                    sg = ep_pool.tile([65, 512], f32, tag="stg", name=f"stg{b}_{g}")
                    nc.scalar.copy(sg[:], scores[b][g][:])
                    stg.append(sg)
                attn = ep_pool.tile([L, L], f32, tag="attn")
                for t in range(NT):
                    src = stg[t // 3][32 * (t % 3) : 32 * (t % 3) + 1, :]
                    nc.sync.dma_start(
                        out=attn[t * 8 : (t + 1) * 8, :],
                        in_=src,
                    )
                if debug:
                    nc.sync.dma_start(out=dbg_attn[b], in_=attn[:])
                ex = ep_pool.tile([L, L], f32, tag="ex")
                nc.scalar.activation(ex[:], attn[:], AF.Exp)
                em = ep_pool.tile([L, L], f32, tag="em")
                nc.vector.tensor_mul(em[:], ex[:], wms[b])

                if debug:
                    nc.sync.dma_start(out=dbg_em[b], in_=em[:])
                rs = ep_pool.tile([L, 1], f32, tag="rs")
                nc.vector.reduce_sum(rs[:], em[:], axis=mybir.AxisListType.X)
                rrecip = ep_pool.tile([L, 1], f32, tag="rrecip")
                nc.vector.reciprocal(rrecip[:], rs[:])

                emt_ps = ps_tr.tile([L, L], f32, tag="emt_ps")
                nc.tensor.transpose(emt_ps[:], em[:], ident[0:L, 0:L])
                emt = ep_pool.tile([L, L], f32r, tag="emt")
                nc.scalar.copy(emt[:], emt_ps[:])
                em_r = ep_pool.tile([L, L], f32r, tag="em_r")
                nc.vector.tensor_copy(em_r[:], em[:])

                cs = ep_pool.tile([L, 1], f32, tag="cs")
                nc.vector.reduce_sum(cs[:], emt[:], axis=mybir.AxisListType.X)
                crecip = ep_pool.tile([L, 1], f32, tag="crecip")
                nc.vector.reciprocal(crecip[:], cs[:])

                # out0[i, d] = rrecip[i] * sum_j em[i, j] q1[j, d]
                o0_ps = ps_ep.tile([L, D], f32, tag="o_ps")
                nc.tensor.matmul(
                    out=o0_ps[:],
                    lhsT=emt[:],
                    rhs=q1ns[b],
                    start=True,
                    stop=True,
                )
                o0_sb = out_pool.tile([L, D], f32, tag="o0_sb")
                nc.scalar.activation(o0_sb[:], o0_ps[:], AF.Copy, scale=rrecip[:])
                nc.sync.dma_start(out=out0[b], in_=o0_sb[:])

                # out1[j, d] = crecip[j] * sum_i em[i, j] q0[i, d]
                o1_ps = ps_ep.tile([L, D], f32, tag="o_ps")
                nc.tensor.matmul(
                    out=o1_ps[:],
                    lhsT=em_r[:],
                    rhs=q0ns[b],
                    start=True,
                    stop=True,
                )
                o1_sb = out_pool.tile([L, D], f32, tag="o1_sb")
                nc.scalar.activation(o1_sb[:], o1_ps[:], AF.Copy, scale=crecip[:])
                nc.sync.dma_start(out=out1[b], in_=o1_sb[:])

    nc.finalize()
    return nc


def _get_nc():
    if "nc" not in _CACHE:
        _CACHE["nc"] = _build_nc()
    return _CACHE["nc"]


def kernel(q0, q1, mask0, mask1, W1, b1, W2, b2, **_unused):
    from concourse.bass_utils import run_bass_kernel_spmd

    q0 = np.asarray(q0, dtype=np.float32)
    q1 = np.asarray(q1, dtype=np.float32)
    W1 = np.ascontiguousarray(np.asarray(W1, dtype=np.float32))
    b1 = np.asarray(b1, dtype=np.float32)
    W2 = np.asarray(W2, dtype=np.float32)
    m0f = np.asarray(mask0).astype(np.float32)
    m1f = np.asarray(mask1).astype(np.float32)

    # host-side prep (tiny): transposed q views, mask outer product, param tiling
    wm_full = (1.0 - m0f[:, :, None] * m1f[:, None, :]).astype(np.float32)
    b1t = np.ascontiguousarray(b1.reshape(OC, 128).T)
    import ml_dtypes

    w2t = np.ascontiguousarray(W2[:, 0].reshape(OC, 128).T).astype(ml_dtypes.bfloat16)
    W1bf = W1.astype(ml_dtypes.bfloat16)

    in_maps = []
    for c in range(N_CORES):
        sl = slice(BPC * c, BPC * (c + 1))
        q0c = np.ascontiguousarray(q0[sl])
        q1c = np.ascontiguousarray(q1[sl])
        in_maps.append(
            {
                "q0n": q0c,
                "q1n": q1c,
                "q0t": np.ascontiguousarray(q0c.transpose(0, 2, 1)).astype(
                    ml_dtypes.bfloat16
                ),
                "q1t": np.ascontiguousarray(q1c.transpose(0, 2, 1)).astype(
                    ml_dtypes.bfloat16
                ),
                "wm": np.ascontiguousarray(wm_full[sl]),
                "w1": W1bf,
                "b1t": b1t,
                "w2t": w2t,
            }
        )

    nc = _get_nc()
    res = run_bass_kernel_spmd(nc, in_maps, core_ids=list(range(N_CORES)))
    out0 = np.concatenate([res.results[c]["out0"] for c in range(N_CORES)], axis=0)
    out1 = np.concatenate([res.results[c]["out1"] for c in range(N_CORES)], axis=0)
    return out0, out1


# revision 27
# speedup vs baseline: 1.0333x; 1.0304x over previous
"""Bahdanau additive attention between two sequences on 8 Trainium2 NeuronCores.

Reference computation (per batch b):
    s0 = q0 @ W1[:D]          # [L, O]
    s1 = q1 @ W1[D:]          # [L, O]
    h[i,j,:]   = tanh(s1[i] + s0[j] + b1)          # [L, L, O]
    attn[i,j]  = h[i,j,:] . W2 (+ b2, drops out of softmax)
    masked     = attn + -1e8 * mask0[i]*mask1[j]
    a_m1 = softmax(masked over j);  a_m2 = softmax(masked over i)
    out0[i] = sum_j a_m1[i,j] q1[j];  out1[j] = sum_i a_m2[i,j] q0[i]

Sharding: data-parallel over batch B=16 -> 2 batches per core; params replicated.

Device strategy per (batch, oc) chunk of 128 output-features:
  PE:  s0T/s1T = W1-chunk.T @ q0T/q1T (fp32), then scores += W2oc.T @ tanh(...)
       as a [128,1]x[128,512] matvec in float32r (full rate at N=512),
       accumulated over the 8 oc chunks in PSUM.
  DVE/GPSIMD: outer-sum tmp[o',(i,j)] = s0T[o',j] + s1T[o',i] via
       step-0 broadcast APs (the dominant vector cost, split across engines).
  ACT: h = tanh(tmp + b1[o'] per-partition bias) - the dominant cost
       (~8.4M transcendentals per core).
Epilogue per batch: exp on ACT, mask multiply (host-precomputed 1-m0*m1),
row/col sums via DVE reduce + PE transpose, normalization folded into the
final bmms as per-partition scales.
"""

import numpy as np

B, L, D, O = 16, 64, 512, 1024
N_CORES = 8
BPC = B // N_CORES  # batches per core
OC = O // 128  # 8 o-chunks
DC = D // 128  # 4 d-chunks
NT = (L * L) // 512  # 8 ij-tiles of 512 per batch

PE_ADD_OC = ()  # outer-adds all on DVE

_CACHE = {}


def _build_nc(debug=False):
    import concourse.mybir as mybir
    import concourse.tile as tile
    from concourse import bacc
    from concourse.masks import make_identity

    f32 = mybir.dt.float32
    f32r = mybir.dt.float32r
    bf16 = mybir.dt.bfloat16
    AF = mybir.ActivationFunctionType

    nc = bacc.Bacc("TRN2", target_bir_lowering=False)

    q0n = nc.dram_tensor("q0n", [BPC, L, D], f32r, kind="ExternalInput")
    q1n = nc.dram_tensor("q1n", [BPC, L, D], f32r, kind="ExternalInput")
    q0t = nc.dram_tensor("q0t", [BPC, D, L], bf16, kind="ExternalInput")
    q1t = nc.dram_tensor("q1t", [BPC, D, L], bf16, kind="ExternalInput")
    wm = nc.dram_tensor("wm", [BPC, L, L], f32, kind="ExternalInput")
    w1 = nc.dram_tensor("w1", [2 * D, O], bf16, kind="ExternalInput")
    b1t = nc.dram_tensor("b1t", [128, OC], f32, kind="ExternalInput")
    w2t = nc.dram_tensor("w2t", [128, OC], bf16, kind="ExternalInput")
    out0 = nc.dram_tensor("out0", [BPC, L, D], f32, kind="ExternalOutput")
    out1 = nc.dram_tensor("out1", [BPC, L, D], f32, kind="ExternalOutput")
    if debug:
        dbg_attn = nc.dram_tensor("dbg_attn", [BPC, L, L], f32, kind="ExternalOutput")
        dbg_em = nc.dram_tensor("dbg_em", [BPC, L, L], f32, kind="ExternalOutput")
        dbg_st = nc.dram_tensor("dbg_st", [BPC, OC, 128, 128], f32, kind="ExternalOutput")
        dbg_h = nc.dram_tensor("dbg_h", [BPC, 128, L, L], f32, kind="ExternalOutput")
        dbg_w1 = nc.dram_tensor("dbg_w1", [128, 2 * DC, O], f32, kind="ExternalOutput")
        dbg_qt = nc.dram_tensor("dbg_qt", [128, BPC, DC, L], f32, kind="ExternalOutput")

    with tile.TileContext(nc) as tc:
        with (
            tc.tile_pool(name="consts", bufs=1) as consts,
            tc.tile_pool(name="qdata", bufs=BPC) as qdata,
            tc.tile_pool(name="st", bufs=3) as st_pool,
            tc.tile_pool(name="tmp", bufs=3) as tmp_pool,
            tc.tile_pool(name="hbuf", bufs=3) as h_pool,
            tc.tile_pool(name="ep", bufs=2) as ep_pool,
            tc.tile_pool(name="outb", bufs=2) as out_pool,
            tc.tile_pool(name="ps_scores", bufs=1, space="PSUM") as ps_scores,
            tc.tile_pool(name="ps_st", bufs=1, space="PSUM") as ps_st,
            tc.tile_pool(name="ps_ep", bufs=1, space="PSUM") as ps_ep,
            tc.tile_pool(name="ps_add", bufs=2, space="PSUM") as ps_add,
            tc.tile_pool(name="ps_tr", bufs=1, space="PSUM") as ps_tr,
        ):
            # ---- constants / weights ----
            # W1 loaded per o-chunk so chunk 0 can start after ~256KB, not 2MB
            w1s = consts.tile([128, OC, 2 * DC, 128], bf16)
            for occ in range(OC):
                nc.sync.dma_start(
                    out=w1s[:, occ, :, :],
                    in_=w1[:, occ * 128 : (occ + 1) * 128].rearrange(
                        "(c p) o -> p c o", p=128
                    ),
                )
            b1s = consts.tile([128, OC], f32)
            nc.sync.dma_start(out=b1s[:], in_=b1t[:])
            w2s = consts.tile([128, OC], bf16)
            nc.sync.dma_start(out=w2s[:], in_=w2t[:])
            ident = consts.tile([128, 128], f32)
            make_identity(nc, ident[:])
            ident_r = consts.tile([128, 128], f32r)
            nc.vector.tensor_copy(ident_r[:], ident[:])

            # ---- per-batch inputs (one DMA per tensor, all batches) ----
            q0ts_all = qdata.tile([128, BPC, DC, L], bf16, tag="q0ts", bufs=1)
            q1ts_all = qdata.tile([128, BPC, DC, L], bf16, tag="q1ts", bufs=1)
            for b in range(BPC):
                nc.sync.dma_start(
                    out=q0ts_all[:, b, :, :],
                    in_=q0t[b].rearrange("(c p) j -> p c j", p=128),
                )
                nc.sync.dma_start(
                    out=q1ts_all[:, b, :, :],
                    in_=q1t[b].rearrange("(c p) j -> p c j", p=128),
                )
            # epilogue-only tensors: load on the gpsimd (SWDGE) queue so they
            # don't compete with W1/q-transpose on the sync queue at startup
            q0ns_all = qdata.tile([L, BPC, D], f32r, tag="q0ns", bufs=1)
            nc.sync.dma_start(out=q0ns_all[:], in_=q0n.rearrange("b l d -> l b d"))
            q1ns_all = qdata.tile([L, BPC, D], f32r, tag="q1ns", bufs=1)
            nc.sync.dma_start(out=q1ns_all[:], in_=q1n.rearrange("b l d -> l b d"))
            wms_all = qdata.tile([L, BPC, L], f32, tag="wms", bufs=1)
            nc.sync.dma_start(out=wms_all[:], in_=wm.rearrange("b l j -> l b j"))
            if debug:
                w1f_d = qdata.tile([128, OC, 2 * DC, 128], f32, tag="w1f_d", bufs=1)
                nc.vector.tensor_copy(w1f_d[:], w1s[:])
                nc.sync.dma_start(out=dbg_w1[:], in_=w1f_d[:])
                qt_d = qdata.tile([128, BPC, DC, L], f32, tag="qt_d", bufs=1)
                nc.vector.tensor_copy(qt_d[:], q0ts_all[:])
                nc.sync.dma_start(out=dbg_qt[:], in_=qt_d[:])
            q0ts = [q0ts_all[:, b, :, :] for b in range(BPC)]
            q1ts = [q1ts_all[:, b, :, :] for b in range(BPC)]
            q0ns = [q0ns_all[:, b, :] for b in range(BPC)]
            q1ns = [q1ns_all[:, b, :] for b in range(BPC)]
            wms = [wms_all[:, b, :] for b in range(BPC)]

            # scores accumulators: 8 matvec chains per batch. The bf16 matmul
            # path supports PSUM column groups {0, 32, 64} (4-byte matmuls are
            # partition-0 only, and base 96 is rejected), so pack 3 chains per
            # bank -> 3 banks per batch.
            scores = [
                [
                    ps_scores.tile(
                        [65, 512], f32, tag=f"scores_g{g}", name=f"scores{b}_g{g}"
                    )
                    for g in range(3)
                ]
                for b in range(BPC)
            ]

            def score_chain(b, t):
                return scores[b][t // 3][32 * (t % 3) : 32 * (t % 3) + 1, :]

            # ---- main pipeline ----
            pending = None
            for b in range(BPC):
                for oc in range(OC):
                    osl = slice(oc * 128, (oc + 1) * 128)
                    # s0T / s1T for this o-chunk: PSUM [128, 128]
                    st_ps = ps_st.tile([128, 128], f32, tag="st_ps")
                    for dc in range(DC):
                        nc.tensor.matmul(
                            out=st_ps[:, 0:64],
                            lhsT=w1s[:, oc, dc, :],
                            rhs=q0ts[b][:, dc, :],
                            start=(dc == 0),
                            stop=(dc == DC - 1),
                        )
                    for dc in range(DC):
                        nc.tensor.matmul(
                            out=st_ps[:, 64:128],
                            lhsT=w1s[:, oc, DC + dc, :],
                            rhs=q1ts[b][:, dc, :],
                            start=(dc == 0),
                            stop=(dc == DC - 1),
                        )
                    st_sb = st_pool.tile([128, 128], f32r, tag="st_sb")
                    if oc % 2 == 0:
                        nc.scalar.copy(st_sb[:], st_ps[:])
                    else:
                        nc.vector.tensor_copy(st_sb[:], st_ps[:])
                    if debug:
                        nc.sync.dma_start(out=dbg_st[b, oc], in_=st_sb[:].bitcast(f32))

                    # outer sum tmp[o', (i, j)] = s0T[o', j] + s1T[o', i].
                    # GPSIMD shares SBUF ports with DVE (no net win), so split
                    # between DVE (tensor_add) and PE (identity-matmul into
                    # PSUM, f32r at full rate; PE has its own SBUF ports).
                    in_j = st_sb[:, 0:64].unsqueeze(1).broadcast_to((128, L, L))
                    in_i = st_sb[:, 64:128].unsqueeze(2).broadcast_to((128, L, L))
                    h3 = h_pool.tile([128, L * L], bf16, tag="h3")
                    if oc in PE_ADD_OC:
                        for n in range(NT):
                            nsl = slice(n * 8, (n + 1) * 8)
                            ptmp = ps_add.tile([128, 512], f32, tag="ptmp")
                            nc.tensor.matmul(
                                out=ptmp[:],
                                lhsT=ident_r[:],
                                rhs=in_j[:, nsl, :],
                                start=True,
                                stop=False,
                            )
                            nc.tensor.matmul(
                                out=ptmp[:],
                                lhsT=ident_r[:],
                                rhs=in_i[:, nsl, :],
                                start=False,
                                stop=True,
                            )
                            nc.scalar.activation(
                                h3[:, n * 512 : (n + 1) * 512],
                                ptmp[:],
                                AF.Tanh,
                                bias=b1s[:, oc : oc + 1],
                            )
                    else:
                        tmp3 = tmp_pool.tile([128, L * L], f32, tag="tmp3")
                        tmp3_3d = tmp3[:].rearrange("p (i j) -> p i j", i=L)
                        nc.vector.tensor_add(tmp3_3d, in_j, in_i)
                        nc.scalar.activation(
                            h3[:], tmp3[:], AF.Tanh, bias=b1s[:, oc : oc + 1]
                        )

                    if debug and oc == 0:
                        h32 = st_pool.tile([128, L * L], f32, tag="h32", name=f"h32_{b}")
                        nc.vector.tensor_copy(h32[:], h3[:])
                        nc.sync.dma_start(out=dbg_h[b], in_=h32[:].rearrange("p (i j) -> p i j", i=L))
                    # matvec for the PREVIOUS chunk (software pipelining: PE
                    # runs chunk c's s-matmuls while ACT finishes tanh(c))
                    if pending is not None:
                        pb, poc, ph = pending
                        for t in range(NT):
                            nc.tensor.matmul(
                                out=score_chain(pb, t),
                                lhsT=w2s[:, poc : poc + 1],
                                rhs=ph[:, t * 512 : (t + 1) * 512],
                                start=(poc == 0),
                                stop=(poc == OC - 1),
                            )
                    pending = (b, oc, h3)

                # flush the last chunk's matvec before batch b's epilogue
                pb, poc, ph = pending
                for t in range(NT):
                    nc.tensor.matmul(
                        out=score_chain(pb, t),
                        lhsT=w2s[:, poc : poc + 1],
                        rhs=ph[:, t * 512 : (t + 1) * 512],
                        start=(poc == 0),
                        stop=(poc == OC - 1),
                    )
                pending = None

                # ---- epilogue for batch b ----
                # PSUM can't be DMA'd: bounce score banks through SBUF (cost is
                # free-dim driven, so copying the full partition range is cheap)
                stg = []
                for g in range(3):
                    sg = ep_pool.tile([65, 512], f32, tag="stg", name=f"stg{b}_{g}")
                    nc.scalar.copy(sg[:], scores[b][g][:])
                    stg.append(sg)
                attn = ep_pool.tile([L, L], f32, tag="attn")
                for t in range(NT):
                    src = stg[t // 3][32 * (t % 3) : 32 * (t % 3) + 1, :]
                    nc.sync.dma_start(
                        out=attn[t * 8 : (t + 1) * 8, :],
                        in_=src,
                    )
                if debug:
                    nc.sync.dma_start(out=dbg_attn[b], in_=attn[:])
                ex = ep_pool.tile([L, L], f32, tag="ex")
                nc.scalar.activation(ex[:], attn[:], AF.Exp)
                em = ep_pool.tile([L, L], f32, tag="em")
                nc.vector.tensor_mul(em[:], ex[:], wms[b])

                if debug:
                    nc.sync.dma_start(out=dbg_em[b], in_=em[:])
                rs = ep_pool.tile([L, 1], f32, tag="rs")
                nc.vector.reduce_sum(rs[:], em[:], axis=mybir.AxisListType.X)
                rrecip = ep_pool.tile([L, 1], f32, tag="rrecip")
                nc.vector.reciprocal(rrecip[:], rs[:])

                emt_ps = ps_tr.tile([L, L], f32, tag="emt_ps")
                nc.tensor.transpose(emt_ps[:], em[:], ident[0:L, 0:L])
                emt = ep_pool.tile([L, L], f32r, tag="emt")
                nc.scalar.copy(emt[:], emt_ps[:])
                em_r = ep_pool.tile([L, L], f32r, tag="em_r")
                nc.vector.tensor_copy(em_r[:], em[:])

                cs = ep_pool.tile([L, 1], f32, tag="cs")
                nc.vector.reduce_sum(cs[:], emt[:], axis=mybir.AxisListType.X)
                crecip = ep_pool.tile([L, 1], f32, tag="crecip")
                nc.vector.reciprocal(crecip[:], cs[:])

                # out0[i, d] = rrecip[i] * sum_j em[i, j] q1[j, d]
                o0_ps = ps_ep.tile([L, D], f32, tag="o_ps")
                nc.tensor.matmul(
                    out=o0_ps[:],
                    lhsT=emt[:],
                    rhs=q1ns[b],
                    start=True,
                    stop=True,
                )
                o0_sb = out_pool.tile([L, D], f32, tag="o0_sb")
                nc.scalar.activation(o0_sb[:], o0_ps[:], AF.Copy, scale=rrecip[:])
                nc.sync.dma_start(out=out0[b], in_=o0_sb[:])

                # out1[j, d] = crecip[j] * sum_i em[i, j] q0[i, d]
                o1_ps = ps_ep.tile([L, D], f32, tag="o_ps")
                nc.tensor.matmul(
                    out=o1_ps[:],
                    lhsT=em_r[:],
                    rhs=q0ns[b],
                    start=True,
                    stop=True,
                )
                o1_sb = out_pool.tile([L, D], f32, tag="o1_sb")
                nc.scalar.activation(o1_sb[:], o1_ps[:], AF.Copy, scale=crecip[:])
                nc.sync.dma_start(out=out1[b], in_=o1_sb[:])

    nc.finalize()
    return nc


def _get_nc():
    if "nc" not in _CACHE:
        _CACHE["nc"] = _build_nc()
    return _CACHE["nc"]


def kernel(q0, q1, mask0, mask1, W1, b1, W2, b2, **_unused):
    from concourse.bass_utils import run_bass_kernel_spmd

    q0 = np.asarray(q0, dtype=np.float32)
    q1 = np.asarray(q1, dtype=np.float32)
    W1 = np.ascontiguousarray(np.asarray(W1, dtype=np.float32))
    b1 = np.asarray(b1, dtype=np.float32)
    W2 = np.asarray(W2, dtype=np.float32)
    m0f = np.asarray(mask0).astype(np.float32)
    m1f = np.asarray(mask1).astype(np.float32)

    # host-side prep (tiny): transposed q views, mask outer product, param tiling
    wm_full = (1.0 - m0f[:, :, None] * m1f[:, None, :]).astype(np.float32)
    b1t = np.ascontiguousarray(b1.reshape(OC, 128).T)
    import ml_dtypes

    w2t = np.ascontiguousarray(W2[:, 0].reshape(OC, 128).T).astype(ml_dtypes.bfloat16)
    W1bf = W1.astype(ml_dtypes.bfloat16)

    in_maps = []
    for c in range(N_CORES):
        sl = slice(BPC * c, BPC * (c + 1))
        q0c = np.ascontiguousarray(q0[sl])
        q1c = np.ascontiguousarray(q1[sl])
        in_maps.append(
            {
                "q0n": q0c,
                "q1n": q1c,
                "q0t": np.ascontiguousarray(q0c.transpose(0, 2, 1)).astype(
                    ml_dtypes.bfloat16
                ),
                "q1t": np.ascontiguousarray(q1c.transpose(0, 2, 1)).astype(
                    ml_dtypes.bfloat16
                ),
                "wm": np.ascontiguousarray(wm_full[sl]),
                "w1": W1bf,
                "b1t": b1t,
                "w2t": w2t,
            }
        )

    nc = _get_nc()
    res = run_bass_kernel_spmd(nc, in_maps, core_ids=list(range(N_CORES)))
    out0 = np.concatenate([res.results[c]["out0"] for c in range(N_CORES)], axis=0)
    out1 = np.concatenate([res.results[c]["out1"] for c in range(N_CORES)], axis=0)
    return out0, out1


# revision 29
# speedup vs baseline: 1.0580x; 1.0239x over previous
"""Bahdanau additive attention between two sequences on 8 Trainium2 NeuronCores.

Reference computation (per batch b):
    s0 = q0 @ W1[:D]          # [L, O]
    s1 = q1 @ W1[D:]          # [L, O]
    h[i,j,:]   = tanh(s1[i] + s0[j] + b1)          # [L, L, O]
    attn[i,j]  = h[i,j,:] . W2 (+ b2, drops out of softmax)
    masked     = attn + -1e8 * mask0[i]*mask1[j]
    a_m1 = softmax(masked over j);  a_m2 = softmax(masked over i)
    out0[i] = sum_j a_m1[i,j] q1[j];  out1[j] = sum_i a_m2[i,j] q0[i]

Sharding: data-parallel over batch B=16 -> 2 batches per core; params replicated.

Device strategy per (batch, oc) chunk of 128 output-features:
  PE:  s0T/s1T = W1-chunk.T @ q0T/q1T (fp32), then scores += W2oc.T @ tanh(...)
       as a [128,1]x[128,512] matvec in float32r (full rate at N=512),
       accumulated over the 8 oc chunks in PSUM.
  DVE/GPSIMD: outer-sum tmp[o',(i,j)] = s0T[o',j] + s1T[o',i] via
       step-0 broadcast APs (the dominant vector cost, split across engines).
  ACT: h = tanh(tmp + b1[o'] per-partition bias) - the dominant cost
       (~8.4M transcendentals per core).
Epilogue per batch: exp on ACT, mask multiply (host-precomputed 1-m0*m1),
row/col sums via DVE reduce + PE transpose, normalization folded into the
final bmms as per-partition scales.
"""

import numpy as np

B, L, D, O = 16, 64, 512, 1024
N_CORES = 8
BPC = B // N_CORES  # batches per core
OC = O // 128  # 8 o-chunks
DC = D // 128  # 4 d-chunks
NT = (L * L) // 512  # 8 ij-tiles of 512 per batch

PE_ADD_OC = ()  # outer-adds all on DVE

_CACHE = {}


def _build_nc(debug=False):
    import concourse.mybir as mybir
    import concourse.tile as tile
    from concourse import bacc
    from concourse.masks import make_identity

    f32 = mybir.dt.float32
    f32r = mybir.dt.float32r
    bf16 = mybir.dt.bfloat16
    AF = mybir.ActivationFunctionType

    nc = bacc.Bacc("TRN2", target_bir_lowering=False)

    q0n = nc.dram_tensor("q0n", [BPC, L, D], f32r, kind="ExternalInput")
    q1n = nc.dram_tensor("q1n", [BPC, L, D], f32r, kind="ExternalInput")
    q0t = nc.dram_tensor("q0t", [BPC, D, L], bf16, kind="ExternalInput")
    q1t = nc.dram_tensor("q1t", [BPC, D, L], bf16, kind="ExternalInput")
    wm = nc.dram_tensor("wm", [BPC, L, L], f32, kind="ExternalInput")
    w1 = nc.dram_tensor("w1", [2 * D, O], bf16, kind="ExternalInput")
    b1t = nc.dram_tensor("b1t", [128, OC], f32, kind="ExternalInput")
    w2t = nc.dram_tensor("w2t", [128, OC], bf16, kind="ExternalInput")
    out0 = nc.dram_tensor("out0", [BPC, L, D], f32, kind="ExternalOutput")
    out1 = nc.dram_tensor("out1", [BPC, L, D], f32, kind="ExternalOutput")
    if debug:
        dbg_attn = nc.dram_tensor("dbg_attn", [BPC, L, L], f32, kind="ExternalOutput")
        dbg_em = nc.dram_tensor("dbg_em", [BPC, L, L], f32, kind="ExternalOutput")
        dbg_st = nc.dram_tensor("dbg_st", [BPC, OC, 128, 128], f32, kind="ExternalOutput")
        dbg_h = nc.dram_tensor("dbg_h", [BPC, 128, L, L], f32, kind="ExternalOutput")
        dbg_w1 = nc.dram_tensor("dbg_w1", [128, 2 * DC, O], f32, kind="ExternalOutput")
        dbg_qt = nc.dram_tensor("dbg_qt", [128, BPC, DC, L], f32, kind="ExternalOutput")

    with tile.TileContext(nc) as tc:
        with (
            tc.tile_pool(name="consts", bufs=1) as consts,
            tc.tile_pool(name="qdata", bufs=BPC) as qdata,
            tc.tile_pool(name="st", bufs=3) as st_pool,
            tc.tile_pool(name="tmp", bufs=3) as tmp_pool,
            tc.tile_pool(name="hbuf", bufs=3) as h_pool,
            tc.tile_pool(name="ep", bufs=2) as ep_pool,
            tc.tile_pool(name="outb", bufs=2) as out_pool,
            tc.tile_pool(name="ps_scores", bufs=1, space="PSUM") as ps_scores,
            tc.tile_pool(name="ps_st", bufs=1, space="PSUM") as ps_st,
            tc.tile_pool(name="ps_ep", bufs=1, space="PSUM") as ps_ep,
            tc.tile_pool(name="ps_add", bufs=2, space="PSUM") as ps_add,
            tc.tile_pool(name="ps_tr", bufs=1, space="PSUM") as ps_tr,
        ):
            # ---- constants / weights ----
            # W1 in two column-chunks: first 2 o-chunks (512KB) land fast so
            # chunk 0 starts early; the rest follows as one efficient DMA
            w1s = consts.tile([128, 2 * DC, O], bf16)
            for lo, hi in ((0, 256), (256, O)):
                nc.sync.dma_start(
                    out=w1s[:, :, lo:hi],
                    in_=w1[:, lo:hi].rearrange("(c p) o -> p c o", p=128),
                )
            b1s = consts.tile([128, OC], f32)
            nc.sync.dma_start(out=b1s[:], in_=b1t[:])
            w2s = consts.tile([128, OC], bf16)
            nc.sync.dma_start(out=w2s[:], in_=w2t[:])
            ident = consts.tile([128, 128], f32)
            make_identity(nc, ident[:])
            ident_r = consts.tile([128, 128], f32r)
            nc.vector.tensor_copy(ident_r[:], ident[:])

            # ---- per-batch inputs (one DMA per tensor, all batches) ----
            q0ts_all = qdata.tile([128, BPC, DC, L], bf16, tag="q0ts", bufs=1)
            q1ts_all = qdata.tile([128, BPC, DC, L], bf16, tag="q1ts", bufs=1)
            for b in range(BPC):
                nc.sync.dma_start(
                    out=q0ts_all[:, b, :, :],
                    in_=q0t[b].rearrange("(c p) j -> p c j", p=128),
                )
                nc.sync.dma_start(
                    out=q1ts_all[:, b, :, :],
                    in_=q1t[b].rearrange("(c p) j -> p c j", p=128),
                )
            # epilogue-only tensors: load on the gpsimd (SWDGE) queue so they
            # don't compete with W1/q-transpose on the sync queue at startup
            q0ns_all = qdata.tile([L, BPC, D], f32r, tag="q0ns", bufs=1)
            nc.sync.dma_start(out=q0ns_all[:], in_=q0n.rearrange("b l d -> l b d"))
            q1ns_all = qdata.tile([L, BPC, D], f32r, tag="q1ns", bufs=1)
            nc.sync.dma_start(out=q1ns_all[:], in_=q1n.rearrange("b l d -> l b d"))
            wms_all = qdata.tile([L, BPC, L], f32, tag="wms", bufs=1)
            nc.sync.dma_start(out=wms_all[:], in_=wm.rearrange("b l j -> l b j"))
            if debug:
                w1f_d = qdata.tile([128, 2 * DC, O], f32, tag="w1f_d", bufs=1)
                nc.vector.tensor_copy(w1f_d[:], w1s[:])
                nc.sync.dma_start(out=dbg_w1[:], in_=w1f_d[:])
                qt_d = qdata.tile([128, BPC, DC, L], f32, tag="qt_d", bufs=1)
                nc.vector.tensor_copy(qt_d[:], q0ts_all[:])
                nc.sync.dma_start(out=dbg_qt[:], in_=qt_d[:])
            q0ts = [q0ts_all[:, b, :, :] for b in range(BPC)]
            q1ts = [q1ts_all[:, b, :, :] for b in range(BPC)]
            q0ns = [q0ns_all[:, b, :] for b in range(BPC)]
            q1ns = [q1ns_all[:, b, :] for b in range(BPC)]
            wms = [wms_all[:, b, :] for b in range(BPC)]

            # scores accumulators: 8 matvec chains per batch. The bf16 matmul
            # path supports PSUM column groups {0, 32, 64} (4-byte matmuls are
            # partition-0 only, and base 96 is rejected), so pack 3 chains per
            # bank -> 3 banks per batch.
            scores = [
                [
                    ps_scores.tile(
                        [65, 512], f32, tag=f"scores_g{g}", name=f"scores{b}_g{g}"
                    )
                    for g in range(3)
                ]
                for b in range(BPC)
            ]

            def score_chain(b, t):
                return scores[b][t // 3][32 * (t % 3) : 32 * (t % 3) + 1, :]

            # ---- main pipeline ----
            pending = None
            for b in range(BPC):
                for oc in range(OC):
                    osl = slice(oc * 128, (oc + 1) * 128)
                    # s0T / s1T for this o-chunk: PSUM [128, 128]
                    st_ps = ps_st.tile([128, 128], f32, tag="st_ps")
                    for dc in range(DC):
                        nc.tensor.matmul(
                            out=st_ps[:, 0:64],
                            lhsT=w1s[:, dc, osl],
                            rhs=q0ts[b][:, dc, :],
                            start=(dc == 0),
                            stop=(dc == DC - 1),
                        )
                    for dc in range(DC):
                        nc.tensor.matmul(
                            out=st_ps[:, 64:128],
                            lhsT=w1s[:, DC + dc, osl],
                            rhs=q1ts[b][:, dc, :],
                            start=(dc == 0),
                            stop=(dc == DC - 1),
                        )
                    st_sb = st_pool.tile([128, 128], f32r, tag="st_sb")
                    if oc % 2 == 0:
                        nc.scalar.copy(st_sb[:], st_ps[:])
                    else:
                        nc.vector.tensor_copy(st_sb[:], st_ps[:])
                    if debug:
                        nc.sync.dma_start(out=dbg_st[b, oc], in_=st_sb[:].bitcast(f32))

                    # outer sum tmp[o', (i, j)] = s0T[o', j] + s1T[o', i].
                    # GPSIMD shares SBUF ports with DVE (no net win), so split
                    # between DVE (tensor_add) and PE (identity-matmul into
                    # PSUM, f32r at full rate; PE has its own SBUF ports).
                    in_j = st_sb[:, 0:64].unsqueeze(1).broadcast_to((128, L, L))
                    in_i = st_sb[:, 64:128].unsqueeze(2).broadcast_to((128, L, L))
                    h3 = h_pool.tile([128, L * L], bf16, tag="h3")
                    if oc in PE_ADD_OC:
                        for n in range(NT):
                            nsl = slice(n * 8, (n + 1) * 8)
                            ptmp = ps_add.tile([128, 512], f32, tag="ptmp")
                            nc.tensor.matmul(
                                out=ptmp[:],
                                lhsT=ident_r[:],
                                rhs=in_j[:, nsl, :],
                                start=True,
                                stop=False,
                            )
                            nc.tensor.matmul(
                                out=ptmp[:],
                                lhsT=ident_r[:],
                                rhs=in_i[:, nsl, :],
                                start=False,
                                stop=True,
                            )
                            nc.scalar.activation(
                                h3[:, n * 512 : (n + 1) * 512],
                                ptmp[:],
                                AF.Tanh,
                                bias=b1s[:, oc : oc + 1],
                            )
                    else:
                        tmp3 = tmp_pool.tile([128, L * L], f32, tag="tmp3")
                        tmp3_3d = tmp3[:].rearrange("p (i j) -> p i j", i=L)
                        nc.vector.tensor_add(tmp3_3d, in_j, in_i)
                        nc.scalar.activation(
                            h3[:], tmp3[:], AF.Tanh, bias=b1s[:, oc : oc + 1]
                        )

                    if debug and oc == 0:
                        h32 = st_pool.tile([128, L * L], f32, tag="h32", name=f"h32_{b}")
                        nc.vector.tensor_copy(h32[:], h3[:])
                        nc.sync.dma_start(out=dbg_h[b], in_=h32[:].rearrange("p (i j) -> p i j", i=L))
                    # matvec for the PREVIOUS chunk (software pipelining: PE
                    # runs chunk c's s-matmuls while ACT finishes tanh(c))
                    if pending is not None:
                        pb, poc, ph = pending
                        for t in range(NT):
                            nc.tensor.matmul(
                                out=score_chain(pb, t),
                                lhsT=w2s[:, poc : poc + 1],
                                rhs=ph[:, t * 512 : (t + 1) * 512],
                                start=(poc == 0),
                                stop=(poc == OC - 1),
                            )
                    pending = (b, oc, h3)

                # flush the last chunk's matvec before batch b's epilogue
                pb, poc, ph = pending
                for t in range(NT):
                    nc.tensor.matmul(
                        out=score_chain(pb, t),
                        lhsT=w2s[:, poc : poc + 1],
                        rhs=ph[:, t * 512 : (t + 1) * 512],
                        start=(poc == 0),
                        stop=(poc == OC - 1),
                    )
                pending = None

                # ---- epilogue for batch b ----
                # PSUM can't be DMA'd: bounce score banks through SBUF (cost is
                # free-dim driven, so copying the full partition range is cheap)
                stg = []
                for g in range(3):
                    sg = ep_pool.tile([65, 512], f32, tag="stg", name=f"stg{b}_{g}")
                    nc.scalar.copy(sg[:], scores[b][g][:])
                    stg.append(sg)
                attn = ep_pool.tile([L, L], f32, tag="attn")
                for t in range(NT):
                    src = stg[t // 3][32 * (t % 3) : 32 * (t % 3) + 1, :]
                    nc.sync.dma_start(
                        out=attn[t * 8 : (t + 1) * 8, :],
                        in_=src,
                    )
                if debug:
                    nc.sync.dma_start(out=dbg_attn[b], in_=attn[:])
                ex = ep_pool.tile([L, L], f32, tag="ex")
                nc.scalar.activation(ex[:], attn[:], AF.Exp)
                em = ep_pool.tile([L, L], f32, tag="em")
                nc.vector.tensor_mul(em[:], ex[:], wms[b])

                if debug:
                    nc.sync.dma_start(out=dbg_em[b], in_=em[:])
                rs = ep_pool.tile([L, 1], f32, tag="rs")
                nc.vector.reduce_sum(rs[:], em[:], axis=mybir.AxisListType.X)
                rrecip = ep_pool.tile([L, 1], f32, tag="rrecip")
                nc.vector.reciprocal(rrecip[:], rs[:])

                emt_ps = ps_tr.tile([L, L], f32, tag="emt_ps")
                nc.tensor.transpose(emt_ps[:], em[:], ident[0:L, 0:L])
                emt = ep_pool.tile([L, L], f32r, tag="emt")
                nc.scalar.copy(emt[:], emt_ps[:])
                em_r = ep_pool.tile([L, L], f32r, tag="em_r")
                nc.vector.tensor_copy(em_r[:], em[:])

                cs = ep_pool.tile([L, 1], f32, tag="cs")
                nc.vector.reduce_sum(cs[:], emt[:], axis=mybir.AxisListType.X)
                crecip = ep_pool.tile([L, 1], f32, tag="crecip")
                nc.vector.reciprocal(crecip[:], cs[:])

                # out0[i, d] = rrecip[i] * sum_j em[i, j] q1[j, d]
                o0_ps = ps_ep.tile([L, D], f32, tag="o_ps")
                nc.tensor.matmul(
                    out=o0_ps[:],
                    lhsT=emt[:],
                    rhs=q1ns[b],
                    start=True,
                    stop=True,
                )
                o0_sb = out_pool.tile([L, D], f32, tag="o0_sb")
                nc.scalar.activation(o0_sb[:], o0_ps[:], AF.Copy, scale=rrecip[:])
                nc.sync.dma_start(out=out0[b], in_=o0_sb[:])

                # out1[j, d] = crecip[j] * sum_i em[i, j] q0[i, d]
                o1_ps = ps_ep.tile([L, D], f32, tag="o_ps")
                nc.tensor.matmul(
                    out=o1_ps[:],
                    lhsT=em_r[:],
                    rhs=q0ns[b],
                    start=True,
                    stop=True,
                )
                o1_sb = out_pool.tile([L, D], f32, tag="o1_sb")
                nc.scalar.activation(o1_sb[:], o1_ps[:], AF.Copy, scale=crecip[:])
                nc.sync.dma_start(out=out1[b], in_=o1_sb[:])

    nc.finalize()
    return nc


def _get_nc():
    if "nc" not in _CACHE:
        _CACHE["nc"] = _build_nc()
    return _CACHE["nc"]


def kernel(q0, q1, mask0, mask1, W1, b1, W2, b2, **_unused):
    from concourse.bass_utils import run_bass_kernel_spmd

    q0 = np.asarray(q0, dtype=np.float32)
    q1 = np.asarray(q1, dtype=np.float32)
    W1 = np.ascontiguousarray(np.asarray(W1, dtype=np.float32))
    b1 = np.asarray(b1, dtype=np.float32)
    W2 = np.asarray(W2, dtype=np.float32)
    m0f = np.asarray(mask0).astype(np.float32)
    m1f = np.asarray(mask1).astype(np.float32)

    # host-side prep (tiny): transposed q views, mask outer product, param tiling
    wm_full = (1.0 - m0f[:, :, None] * m1f[:, None, :]).astype(np.float32)
    b1t = np.ascontiguousarray(b1.reshape(OC, 128).T)
    import ml_dtypes

    w2t = np.ascontiguousarray(W2[:, 0].reshape(OC, 128).T).astype(ml_dtypes.bfloat16)
    W1bf = W1.astype(ml_dtypes.bfloat16)

    in_maps = []
    for c in range(N_CORES):
        sl = slice(BPC * c, BPC * (c + 1))
        q0c = np.ascontiguousarray(q0[sl])
        q1c = np.ascontiguousarray(q1[sl])
        in_maps.append(
            {
                "q0n": q0c,
                "q1n": q1c,
                "q0t": np.ascontiguousarray(q0c.transpose(0, 2, 1)).astype(
                    ml_dtypes.bfloat16
                ),
                "q1t": np.ascontiguousarray(q1c.transpose(0, 2, 1)).astype(
                    ml_dtypes.bfloat16
                ),
                "wm": np.ascontiguousarray(wm_full[sl]),
                "w1": W1bf,
                "b1t": b1t,
                "w2t": w2t,
            }
        )

    nc = _get_nc()
    res = run_bass_kernel_spmd(nc, in_maps, core_ids=list(range(N_CORES)))
    out0 = np.concatenate([res.results[c]["out0"] for c in range(N_CORES)], axis=0)
    out1 = np.concatenate([res.results[c]["out1"] for c in range(N_CORES)], axis=0)
    return out0, out1
